# revision 2
# baseline (speedup 1.0000x reference)
"""DynaConvBlock Trainium2 kernel (8 NeuronCores).

Strategy (all heavy traffic bf16; fp32 PSUM accumulation):
  Phase H (hypernet): every core redundantly computes the small layers
    (h1, z, h2) for all 32 samples; the big ks matmul (lat -> 33088
    cols) is sharded over output columns (4096 + 320 bias cols per
    core). The per-core weight slice vwT is column-PERMUTED on the host
    ([kin | bias | kout], conv-ready orientation) so the per-sample conv
    kernels later load straight into lhsT layout with no on-chip
    transposes:
      kin block  col i = f*32 + h   <-> ks col (32c + h)*64 + f
      kout block col i = h*8  + o   <-> ks col KIN + (8c + o)*256 + h
    The b2_b2 bias is folded post-exchange via a host-precomputed tile.
  Split exchange: AllToAll #1 (kin + bias) fires while the kout half of
    vw is still streaming; AllToAll #2 (kout) follows. x prefetch and
    conv-weight prep fill the collective windows; tile_wait_until floors
    order the DMA queue so the exchange stores are never starved.
  Phase C: software-pipelined conv -- conv1 (K=64, PE row groups per
    sample) runs up to LOOK chunks ahead on exchange #1 alone; conv2
    (M=64, both samples packed in one PSUM bank via PE col groups) joins
    once exchange #2 lands. Epilogues alternate ScalarE/VectorE; bf16
    output, host upcasts to f32.
"""
import numpy as np

import concourse.bass as bass
import concourse.mybir as mybir
import concourse.tile as tile
from concourse import bacc
from concourse import bass_utils

# problem dims
B, FIN, FOUT, FH, H, W = 32, 64, 64, 256, 128, 128
LAT = 512
HW = H * W                      # 16384
KIN, KOUT = FH * FIN, FOUT * FH  # 16384, 16384
OUT_SZ = KIN + KOUT + FH + FOUT  # 33088
B1H, B2H = 512, 1024

NCORES = 8
SPC = B // NCORES               # 4 samples per core
KIN_SH = KIN // NCORES          # 2048 ks-cols per core (k_in part)
KOUT_SH = KOUT // NCORES        # 2048 (k_out part)
XCH = KIN_SH + KOUT_SH          # 4096 exchanged cols per core
BIAS_N = FH + FOUT              # 320 bias cols (replicated)
NBC = XCH + BIAS_N              # 4416 bnc cols
NA = KIN_SH + BIAS_N            # 2368: exchange #1 (kin + bias)
NB = KOUT_SH                    # 2048: exchange #2 (kout)
KTOT = LAT + B2H                # 1536 contraction for ks matmul
NKC = KTOT // 128               # 12 k-chunks

F32 = mybir.dt.float32
BF16 = mybir.dt.bfloat16
RELU = mybir.ActivationFunctionType.Relu
IDENT = mybir.ActivationFunctionType.Identity
ADD = mybir.AluOpType.add
MAX = mybir.AluOpType.max

_CACHED_NC = None


def _build(single_core=False):
    nc = bacc.Bacc("TRN2", target_bir_lowering=False, debug=False,
                   num_devices=1 if single_core else NCORES)

    x_d = nc.dram_tensor("x", [SPC * FIN, HW], BF16, kind="ExternalInput").ap()
    # all small hypernet weights, host-packed k-chunked:
    # [latT 128 | w1 2048 | ws 2048 | w2 2048 | v1 4096]
    sw_d = nc.dram_tensor("sw", [128, 10368], BF16, kind="ExternalInput").ap()
    vwT_d = nc.dram_tensor("vwT", [KTOT, NBC], BF16, kind="ExternalInput").ap()
    # small-layer biases, host-pre-broadcast to [32, 512|512|1024]
    bsm_d = nc.dram_tensor("bsm", [B, B1H + LAT + B2H], BF16,
                           kind="ExternalInput").ap()
    bks_d = nc.dram_tensor("bks", [128, 387], BF16, kind="ExternalInput").ap()
    out_d = nc.dram_tensor("out", [SPC * FOUT, HW], BF16, kind="ExternalOutput").ap()

    with tile.TileContext(nc) as tc:
        _emit(nc, tc, x_d, sw_d, vwT_d, bsm_d, bks_d, out_d,
              single_core=single_core)
    nc.compile()
    return nc


def _emit(nc, tc, x_d, sw_d, vwT_d, bsm_d, bks_d, out_d, single_core=False):
    from contextlib import ExitStack
    from concourse.masks import make_identity
    ctx = ExitStack()
    with ctx:
        const = ctx.enter_context(tc.tile_pool(name="const", bufs=1))
        identb = const.tile([128, 128], BF16)
        make_identity(nc, identb[:])

        # tiny dummy AllToAll issued first: absorbs the collective-stream
        # bootstrap barrier (~40us observed) before the real exchanges
        dpool = ctx.enter_context(tc.tile_pool(name="dummy", bufs=1, space="DRAM"))
        dumA = dpool.tile([B, 16], BF16, name="dumA")
        dumB = dpool.tile([B, 16], BF16, name="dumB")
        if not single_core:
            nc.gpsimd.collective_compute(
                "AllToAll", mybir.AluOpType.bypass,
                replica_groups=[list(range(NCORES))],
                ins=[dumA.opt()], outs=[dumB.opt()],
            )

        # ---------------- Phase H: hypernet ----------------
        hsb = ctx.enter_context(tc.tile_pool(name="hsb", bufs=1))
        dram = ctx.enter_context(tc.tile_pool(name="dram", bufs=1, space="DRAM"))
        bncA_in = dram.tile([B, NA], BF16, name="bncA_in")
        bncA_out = dram.tile([B, NA], BF16, name="bncA_out")
        bncB_in = dram.tile([B, NB], BF16, name="bncB_in")
        bncB_out = dram.tile([B, NB], BF16, name="bncB_out")

        # vw pool allocated first so hw_w can release before it (stack order)
        hwv = tc.alloc_tile_pool(name="hw_vw", bufs=1)
        hww = tc.alloc_tile_pool(name="hw_w", bufs=1)

        # lhsT holder for the big matmul: k-chunks 0-3 = zT, 4-11 = h2T
        zh2T = hsb.tile([128, 32 * NKC], BF16, name="zh2T")

        # --- part A: small layers (own psum pools, closed before big-mm)
        with tc.tile_pool(name="hps", bufs=2, space="PSUM") as hps, \
             tc.tile_pool(name="tps", bufs=2, space="PSUM") as tps:

            sw = hww.tile([128, 10368], BF16, name="sw")
            nc.sync.dma_start(sw[:], sw_d)
            latT = sw[:, 0:128]
            w1sb = sw[:, 128:2176]
            wssb = sw[:, 2176:4224]
            w2sb = sw[:, 4224:6272]
            v1sb = sw[:, 6272:10368]

            # small-layer biases (host pre-broadcast): one DMA
            bsm = hsb.tile([B, B1H + LAT + B2H], BF16, name="bsm")
            nc.sync.dma_start(bsm[:], bsm_d)
            bb1 = bsm[:, 0:B1H]
            bb2 = bsm[:, B1H:B1H + LAT]
            bb3 = bsm[:, B1H + LAT:]

            # vw streaming, split kin+bias (A) then kout (B) so the first
            # exchange can fire while B still streams. The B loads are
            # emitted AFTER the A store (chained; they overlap exchange #1).
            vwsA, vwsB = [], []
            for j in range(NKC):
                vw = hwv.tile([128, NA], BF16, tag=f"vwA{j}")
                nc.sync.dma_start(vw[:], vwT_d[128 * j:128 * (j + 1), 0:NA])
                vwsA.append(vw)
            # host-precomputed b2_b2 bias tiles in conv-kernel layout:
            # cols 0-255 = kiT bias (m0|m1), 256-383 = koT bias (m0|m1),
            # 384-385 = b_in bias (m0|m1), 386 = b_out bias
            cb = hsb.tile([128, 387], BF16, name="cb")
            nc.sync.dma_start(cb[:], bks_d)

            def layer_T(src32, dst, coloff, nch):
                # transpose [32, 128*nch] bf16 tile -> dst[:, coloff+32j]
                for j in range(nch):
                    tp = tps.tile([128, B], BF16, tag="tp")
                    nc.tensor.transpose(tp[:], src32[:, 128 * j:128 * (j + 1)],
                                        identb[:B, :B])
                    nc.vector.tensor_copy(dst[:, coloff + 32 * j:coloff + 32 * (j + 1)],
                                          tp[:])

            # h1 = relu(lat @ w1.T + b1b1)  -> [32, 512]
            h1 = hsb.tile([B, B1H], BF16, name="h1")
            p1 = hps.tile([B, B1H], F32, tag="lp")
            for j in range(4):
                nc.tensor.matmul(p1[:], latT[:, 32 * j:32 * (j + 1)],
                                 w1sb[:, 512 * j:512 * (j + 1)],
                                 start=(j == 0), stop=(j == 3))
            nc.vector.tensor_add(h1[:], p1[:], bb1)
            nc.scalar.activation(h1[:], h1[:], RELU)
            h1T = hsb.tile([128, 128], BF16, name="h1T")
            layer_T(h1, h1T, 0, 4)

            # z = lat @ ws.T + h1 @ w2.T + b1b2 -> [32, 512]
            z = hsb.tile([B, LAT], BF16, name="z")
            p2 = hps.tile([B, LAT], F32, tag="lp")
            for j in range(4):
                nc.tensor.matmul(p2[:], latT[:, 32 * j:32 * (j + 1)],
                                 wssb[:, 512 * j:512 * (j + 1)],
                                 start=(j == 0), stop=False)
            for j in range(4):
                nc.tensor.matmul(p2[:], h1T[:, 32 * j:32 * (j + 1)],
                                 w2sb[:, 512 * j:512 * (j + 1)],
                                 start=False, stop=(j == 3))
            nc.vector.tensor_add(z[:], p2[:], bb2)
            layer_T(z, zh2T, 0, 4)

            # h2 = relu(z @ v1.T + b2b1) -> [32, 1024]
            h2 = hsb.tile([B, B2H], BF16, name="h2")
            for half in range(2):
                p3 = hps.tile([B, 512], F32, tag="lp")
                for j in range(4):
                    nc.tensor.matmul(
                        p3[:], zh2T[:, 32 * j:32 * (j + 1)],
                        v1sb[:, 1024 * j + 512 * half:1024 * j + 512 * (half + 1)],
                        start=(j == 0), stop=(j == 3))
                hs = h2[:, 512 * half:512 * (half + 1)]
                nc.vector.tensor_add(hs, p3[:], bb3[:, 512 * half:512 * (half + 1)])
                nc.scalar.activation(hs, hs, RELU)
            layer_T(h2, zh2T, 128, 8)
        hww.release()

        # --- part B: big ks matmul with 8 PSUM banks (chunk-parallel);
        # kin+bias chunks first (-> exchange #1), kout chunks second
        ksbA = hsb.tile([B, NA], BF16, name="ksbA")
        ksbB = hsb.tile([B, NB], BF16, name="ksbB")

        def mm_chunks(kps, vws, ksb, chunks, store_to=None):
            for ci, (off, n) in enumerate(chunks):
                pk = kps.tile([B, 512], F32, tag="kp", name="pk")
                for j in range(NKC):
                    nc.tensor.matmul(pk[:, :n], zh2T[:, 32 * j:32 * (j + 1)],
                                     vws[j][:, off:off + n],
                                     start=(j == 0), stop=(j == NKC - 1))
                # b2_b2 bias is folded in post-exchange, so this is a pure
                # PSUM->SBUF cast
                if ci % 2 == 0:
                    nc.vector.tensor_copy(ksb[:, off:off + n], pk[:, :n])
                else:
                    nc.scalar.activation(ksb[:, off:off + n], pk[:, :n], IDENT)
                if store_to is not None:
                    # per-chunk store: each slice ships as soon as its
                    # epilogue lands instead of waiting for the full tile
                    nc.sync.dma_start(store_to[:, off:off + n],
                                      ksb[:, off:off + n])

        kps = tc.alloc_tile_pool(name="kps", space="PSUM", bufs=8)
        mm_chunks(kps, vwsA, ksbA,
                  [(512 * i, 512) for i in range(4)] + [(KIN_SH, BIAS_N)])
        nc.sync.dma_start(bncA_in[:], ksbA[:])
        # vwB loads: scheduling-time floor keeps them out of the DMA queue
        # until the A store has gone out (they then overlap exchange #1)
        with tc.tile_wait_until(0.032):
            for j in range(NKC):
                vw = hwv.tile([128, NB], BF16, tag=f"vwB{j}", name="vwB")
                nc.sync.dma_start(vw[:], vwT_d[128 * j:128 * (j + 1), NA:NBC])
                vwsB.append(vw)
        if single_core:
            nc.sync.dma_start(bncA_out[:], bncA_in[:])
        else:
            nc.gpsimd.collective_compute(
                "AllToAll", mybir.AluOpType.bypass,
                replica_groups=[list(range(NCORES))],
                ins=[bncA_in.opt()], outs=[bncA_out.opt()],
            )
        mm_chunks(kps, vwsB, ksbB, [(512 * i, 512) for i in range(4)])
        nc.sync.dma_start(bncB_in[:], ksbB[:])
        if single_core:
            nc.sync.dma_start(bncB_out[:], bncB_in[:])
        else:
            nc.gpsimd.collective_compute(
                "AllToAll", mybir.AluOpType.bypass,
                replica_groups=[list(range(NCORES))],
                ins=[bncB_in.opt()], outs=[bncB_out.opt()],
            )
        kps.release()
        hwv.release()

        # post-exchange prep, split across SP/ACT DMA queues
        dma_engines = [nc.sync, nc.scalar]
        _dmas = [0]

        def prep_dma(dst, src):
            eng = dma_engines[_dmas[0] % 2]
            _dmas[0] += 1
            eng.dma_start(dst, src)

        cb32 = hsb.tile([128, 3], F32, name="cb32")
        nc.vector.tensor_copy(cb32[:], cb[:, 384:387])
        cpar = ctx.enter_context(tc.tile_pool(name="cpar", bufs=1))
        kiT = [None, None]               # [p]: [128, 256] (m-blocks)
        koT = [[None, None], [None, None]]  # [p][sh]: [128, 128] (m-blocks)
        bIn32 = []
        # floor keeps prep requests behind the B store in the DMA queue
        with tc.tile_wait_until(0.058):
            # conv1 weights first (gate conv1):
            # kiT[p][64*sh+f, 128*m+32*q+h] -- one DMA per (p, sh)
            for p in range(2):
                ki = cpar.tile([128, 256], BF16, tag=f"kiT{p}")
                for sh in range(2):
                    base = 2 * p + sh
                    src = bncA_out[base:base + 29:4, 0:KIN_SH] \
                        .rearrange("(m q) (f h) -> f m q h", q=4, h=32)
                    prep_dma(ki[64 * sh:64 * sh + 64, :]
                             .rearrange("f (m q h) -> f m q h", m=2, h=32),
                             src)
                nc.vector.tensor_add(ki[:], ki[:], cb[:, 0:256])
                kiT[p] = ki
            # bias extraction (rows 0:4 of bncA_out = shard 0's copy)
            for m in range(2):
                bInb = hsb.tile([128, SPC], BF16, name=f"bInb{m}")
                prep_dma(bInb[:],
                         bncA_out[0:SPC,
                                  KIN_SH + 128 * m:KIN_SH + 128 * (m + 1)]
                         .rearrange("s h -> h s"))
                bi = hsb.tile([128, SPC], F32, name=f"bIn32{m}")
                nc.vector.tensor_scalar_add(bi[:], bInb[:], cb32[:, m:m + 1])
                bIn32.append(bi)
            bOutb = hsb.tile([128, 2], BF16, name="bOutb")
            for p in range(2):
                for sh in range(2):
                    src = bncA_out[2 * p + sh:2 * p + sh + 1,
                                   KIN_SH + 256:KIN_SH + 320] \
                        .rearrange("u o -> (u o)")
                    prep_dma(bOutb[64 * sh:64 * sh + 64, p:p + 1],
                             src[:, None])
            bOut32 = hsb.tile([128, 2], F32, name="bOut32")
            nc.vector.tensor_scalar_add(bOut32[:], bOutb[:], cb32[:, 2:3])

        # conv2 weights from exchange #2 (pair 0 first):
        # koT[p][sh][h, 64*m+8*r+o] -- one DMA per (p, sh)
        for p in range(2):
            for sh in range(2):
                base = 2 * p + sh
                ko = cpar.tile([128, 128], BF16, tag=f"koT{p}{sh}")
                src_all = bncB_out[base:base + 29:4, 0:KOUT_SH] \
                    .rearrange("r (h o) -> h r o", o=8)
                for m in range(2):
                    prep_dma(ko[:, 64 * m:64 * (m + 1)]
                             .rearrange("h (r o) -> h r o", o=8),
                             src_all[128 * m:128 * (m + 1), :, :])
                # gpsimd: DVE is busy with conv1-prefix epilogues here
                nc.gpsimd.tensor_add(ko[:], ko[:], cb[:, 256:384])
                koT[p][sh] = ko

        # x prefetch with staged floors: g0/g1 slot in right after the A
        # store; later groups yield to storeB + prep; pair 1 yields to koT
        cx = ctx.enter_context(tc.tile_pool(name="cx", bufs=8))
        xws = [[None] * (HW // 2048) for _ in range(2)]
        for p in range(2):
            for g in range(HW // 2048):
                if p == 0:
                    fl = 0.048 if g == 0 else (0.0485 if g == 1 else 0.063)
                else:
                    fl = 0.085
                with tc.tile_wait_until(fl):
                    xw = cx.tile([128, 2048], BF16, tag=f"x{p}",
                                 name=f"xw{p}_{g}", bufs=6 if p == 0 else 4)
                    nc.sync.dma_start(
                        xw[:],
                        x_d[128 * p:128 * (p + 1), 2048 * g:2048 * (g + 1)])
                    xws[p][g] = xw

        # -------- Phase C: software-pipelined conv (conv1 runs LOOKAHEAD
        # chunks ahead of conv2, so conv1 proceeds on exchange #1 alone
        # while exchange #2 is still in flight) --------
        LOOK = 16
        NCH_T = HW // 512 * 2        # 64 chunks across both pairs
        chp = ctx.enter_context(tc.tile_pool(name="chp", bufs=LOOK + 2))
        co = ctx.enter_context(tc.tile_pool(name="co", bufs=3))
        cps = ctx.enter_context(tc.tile_pool(name="cps", bufs=1, space="PSUM"))
        ops = ctx.enter_context(tc.tile_pool(name="ops", bufs=2, space="PSUM"))

        hq = {}
        state = {"ot": None, "opw": None}

        def conv1_chunk(i):
            p, ci = divmod(i, HW // 512)
            g, cc = divmod(ci, 4)
            xs = xws[p][g][:, 512 * cc:512 * (cc + 1)]
            hts = [[None, None], [None, None]]
            for m in range(2):
                for sh in range(2):
                    hp = cps.tile([128, 512], F32, tag=f"hp{m}{sh}",
                                  name="hp", bufs=2 if m == 0 else 1)
                    nc.tensor.matmul(
                        hp[:],
                        kiT[p][64 * sh:64 * sh + 64, 128 * m:128 * (m + 1)],
                        xs[64 * sh:64 * sh + 64, :], start=True, stop=True)
                    ht = chp.tile([128, 512], BF16, tag=f"h{m}{sh}", name="ht")
                    s = 2 * p + sh
                    if m == 0:
                        nc.scalar.activation(ht[:], hp[:], RELU,
                                             bias=bIn32[m][:, s:s + 1])
                    else:
                        nc.vector.tensor_scalar(ht[:], hp[:],
                                                bIn32[m][:, s:s + 1],
                                                0.0, ADD, MAX)
                    hts[m][sh] = ht
            hq[i] = hts

        def conv2_chunk(i):
            p, ci = divmod(i, HW // 512)
            g, cc = divmod(ci, 4)
            if cc == 0:
                state["ot"] = co.tile([128, 2048], BF16, tag="ot", name="ot")
            ot = state["ot"]
            opw = ops.tile([128, 512], F32, tag="op", name="opw")
            hts = hq.pop(i)
            for sh in range(2):
                for m in range(2):
                    nc.tensor.matmul(
                        opw[64 * sh:64 * sh + 64, :],
                        koT[p][sh][:, 64 * m:64 * (m + 1)], hts[m][sh][:],
                        start=(m == 0), stop=(m == 1))
            ots = ot[:, 512 * cc:512 * (cc + 1)]
            if i % 2 == 0:
                nc.scalar.activation(ots, opw[:], IDENT,
                                     bias=bOut32[:, p:p + 1])
            else:
                nc.vector.tensor_scalar_add(ots, opw[:],
                                            bOut32[:, p:p + 1])
            if cc == 3:
                if i == NCH_T - 1:
                    # split the final store so the kernel tail only waits
                    # on the last chunk's epilogue
                    for q in range(4):
                        nc.sync.dma_start(
                            out_d[128 * p:128 * (p + 1),
                                  2048 * g + 512 * q:2048 * g + 512 * (q + 1)],
                            ot[:, 512 * q:512 * (q + 1)])
                else:
                    nc.sync.dma_start(
                        out_d[128 * p:128 * (p + 1), 2048 * g:2048 * (g + 1)],
                        ot[:])

        for i in range(LOOK):
            conv1_chunk(i)
        for i in range(NCH_T):
            if i + LOOK < NCH_T:
                conv1_chunk(i + LOOK)
            conv2_chunk(i)


def _prep_inputs(inputs):
    import ml_dtypes
    bf = ml_dtypes.bfloat16
    f = {k: np.asarray(v, dtype=np.float32) for k, v in inputs.items()}
    x = f["x"]                      # [32, 64, 128, 128]
    lat = f["lat"]                  # [32, 512]

    def kchunk(wT):                 # [K, N] -> [128, (K//128)*N], col blk j=k
        K_, N_ = wT.shape
        return wT.reshape(K_ // 128, 128, N_).transpose(1, 0, 2) \
            .reshape(128, -1)

    sw = np.concatenate([
        kchunk(lat.T), kchunk(f["b1_w1"].T), kchunk(f["b1_ws"].T),
        kchunk(f["b1_w2"].T), kchunk(f["b2_w1"].T)], axis=1)
    b2wsT = f["b2_ws"].T            # [512, 33088]
    b2w2T = f["b2_w2"].T            # [1024, 33088]
    b2b2 = f["b2_b2"]

    # b2_b2 bias tiles in post-exchange conv-kernel layout (see _emit)
    cbias = np.zeros((128, 387), np.float32)
    f64 = np.arange(64)
    f128 = np.arange(128)
    for m in range(2):
        blk = b2b2[(128 * m + f128[None, :]) * 64 + f64[:, None]]  # [64, 128]
        cbias[0:64, 128 * m:128 * (m + 1)] = blk
        cbias[64:128, 128 * m:128 * (m + 1)] = blk
        cbias[:, 256 + 64 * m:256 + 64 * (m + 1)] = \
            b2b2[KIN + f64[None, :] * 256 + 128 * m + f128[:, None]]
        cbias[:, 384 + m] = b2b2[KIN + KOUT + 128 * m + f128]
    bo = b2b2[KIN + KOUT + 256 + f64]
    cbias[0:64, 386] = bo
    cbias[64:128, 386] = bo

    bsm = np.broadcast_to(
        np.concatenate([f["b1_b1"], f["b1_b2"], f["b2_b1"]])[None, :],
        (B, B1H + LAT + B2H))
    shared = {
        "sw": np.ascontiguousarray(sw).astype(bf),
        "bsm": np.ascontiguousarray(bsm).astype(bf), "bks": cbias.astype(bf),
    }
    i2 = np.arange(XCH // 2)
    in_maps = []
    for c in range(NCORES):
        kin_cols = (32 * c + (i2 % 32)) * 64 + (i2 // 32)
        kout_cols = KIN + (8 * c + (i2 % 8)) * 256 + (i2 // 8)
        bias_cols = np.arange(KIN + KOUT, OUT_SZ)
        # layout [kin | bias | kout] so the kin+bias prefix exchanges first
        cols = np.concatenate([kin_cols, bias_cols, kout_cols])
        vwT = np.ascontiguousarray(
            np.concatenate([b2wsT[:, cols], b2w2T[:, cols]], axis=0)).astype(bf)
        xc = np.ascontiguousarray(
            x[SPC * c:SPC * (c + 1)].reshape(SPC * FIN, HW)).astype(bf)
        in_maps.append({**shared, "x": xc, "vwT": vwT})
    return in_maps


def _run(inputs, trace=False):
    global _CACHED_NC
    if _CACHED_NC is None:
        _CACHED_NC = _build()
    in_maps = _prep_inputs(inputs)
    kw = {}
    if trace:
        kw = dict(trace=True, trace_cores=[0])
    res = bass_utils.run_bass_kernel_spmd(
        _CACHED_NC, in_maps, core_ids=list(range(NCORES)), **kw)
    outs = []
    for c in range(NCORES):
        oc = np.asarray(res.results[c]["out"], dtype=np.float32) \
            .reshape(SPC, FOUT, H, W)
        outs.append(oc)
    full = np.concatenate(outs, axis=0)
    return full, res


def kernel(**inputs) -> np.ndarray:
    out, _ = _run(inputs, trace=False)
    return out

